# revision 6
# baseline (speedup 1.0000x reference)
"""Trainium2 Bass kernel for nn_FMNet pixel-shuffle + sigmoid.

reference:  x = FV[:, 64:, :, :]                                 # [B, 64, 64, 64]
            out[b, 8i+r, 8j+c] = sigmoid(x[b, 8r+c, i, j])       # [B, 1, 512, 512]

Per core (4 batches, pure data-parallel over batch):
  - 8 SWDGE loads (gpsimd Q7 generator) of 512 KiB: per (batch, channel-half),
    partition = (b, i2) spatial-row-pair, 512-byte contiguous HBM chunks.
    SWDGE keeps the load descriptor generation off the single shared HWDGE.
  - 8 fused ScalarE ACTIVATE(Sigmoid) ops [128 x 1024] whose strided input AP
    performs the (c', j) -> (j*8 + c') pixel-shuffle interleave in the same
    pass (measured ~2 ns/elem; DVE/GpSimd strided copies are ~4.4 ns/elem).
  - 16 HWDGE stores (SP engine, now otherwise idle) of 256 KiB: per
    (batch, r-quarter), 4 KiB contiguous HBM chunks, issued as soon as the
    two ACTs they depend on are done - keeps the store tail short.
"""

import os
import sys

if "/opt/trn_rl_repo" not in sys.path:
    sys.path.insert(0, "/opt/trn_rl_repo")

import numpy as np

import concourse.bass as bass
from concourse import mybir
from concourse.bass_utils import run_bass_kernel_spmd

N_CORES = 8
B = 32
B_LOC = B // N_CORES   # 4
H = W = 512
S = 64
NG = 8                 # channel groups (r)

LAST_EXEC_NS = None

_cached_nc = None


def _install_trace_hook():
    """Best-effort NTFF hook so BASS_TRACE=1 yields exec_time_ns."""
    try:
        import types

        import antenv

        try:
            from antenv.axon_hooks import get_axon_ntff_profile_hook  # noqa: F401

            return
        except ImportError:
            pass
        mod = types.ModuleType("antenv.axon_hooks")
        _state = {"hook": None}
        mod.set_axon_ntff_profile_hook = lambda h: _state.__setitem__("hook", h)
        mod.get_axon_ntff_profile_hook = lambda: _state["hook"]
        sys.modules["antenv.axon_hooks"] = mod
        antenv.axon_hooks = mod
        from trn_agent_boot.trn_boot import _ntff_profile_via_ctypes

        mod.set_axon_ntff_profile_hook(
            _ntff_profile_via_ctypes("/opt/axon/libaxon_pjrt.so")
        )
    except Exception:
        pass


def _build_nc():
    import contextlib

    F32 = mybir.dt.float32
    nc = bass.Bass("TRN2", num_devices=N_CORES)
    FV = nc.declare_dram_parameter("FV", [B_LOC, 128, S, S], F32, isOutput=False)
    OUT = nc.declare_dram_parameter("OUT", [B_LOC, W, H], F32, isOutput=True)

    # TIN_h[p=(b,i2), (c32, ip, j)] for channel half h (c32 = c' within half)
    tin = [nc.alloc_sbuf_tensor(f"tin{h}", [128, 4096], F32) for h in range(2)]
    # TOUT_h[p=(b,i2), (ip, r4, q)] for r half h
    tout = [nc.alloc_sbuf_tensor(f"tout{h}", [128, 4096], F32) for h in range(2)]

    fv = FV[:]
    out = OUT[:]

    scratch = nc.alloc_sbuf_tensor("scratch", [1, 8], F32)

    with contextlib.ExitStack() as stack:
        block = stack.enter_context(nc.Block())
        # load waves: q0 = c'[0,8) (feeds ACT_0), q1 = c'[8,32), h1 = c'[32,64)
        sem_q0 = stack.enter_context(nc.semaphore("sem_q0"))
        sem_q1 = stack.enter_context(nc.semaphore("sem_q1"))
        sem_h1 = stack.enter_context(nc.semaphore("sem_h1"))
        sem_act = stack.enter_context(nc.semaphore("sem_act"))
        sem_out = stack.enter_context(nc.semaphore("sem_out"))

        def load(sync, b, clo, chi, sem):
            # channels [64+clo, 64+chi): [c, i2, (ip j)]; 512 B HBM chunks
            src = fv[b, 64 + clo : 64 + chi]
            src = src.rearrange("c (i2 ip) j -> i2 c (ip j)", ip=2)
            h, c0 = divmod(clo, 32)
            dst = tin[h].ap()[32 * b : 32 * b + 32, 128 * c0 : 128 * (c0 + chi - clo)]
            sync.dma_start(out=dst, in_=src).then_inc(sem, 16)

        @block.sync
        def _(sync: bass.BassEngine):
            for b in range(B_LOC):
                load(sync, b, 0, 8, sem_q0)
            for b in range(B_LOC):
                load(sync, b, 8, 32, sem_q1)
            for b in range(B_LOC):
                load(sync, b, 32, 64, sem_h1)
            for rq in range(4):          # r-quarter: r in {2rq, 2rq+1}
                h, k = divmod(rq, 2)     # tout half h, quarter k within half
                sync.wait_ge(sem_act, 2 * (rq + 1))
                for b in range(B_LOC):
                    # dest rows 16*i2 + 8*ip + (2rq + r2), cols q
                    dst = out[b].rearrange(
                        "(i2 ip rr r2) q -> i2 ip rr (r2 q)", i2=32, ip=2, rr=4
                    )[:, :, rq, :]  # [32, 2, 1024]
                    src = tout[h].ap().rearrange(
                        "p (ip r2 v) -> p ip r2 v", ip=2, r2=2
                    )[32 * b : 32 * b + 32, :, k, :]  # [32, 2, 1024]
                    sync.dma_start(out=dst, in_=src).then_inc(sem_out, 16)
            sync.wait_ge(sem_out, 16 * 4 * B_LOC)

        @block.scalar
        def _(scalar: bass.BassEngine):
            # dummy op to pull ACT_TABLE_LOAD (sigmoid) off the critical path
            scalar.activation(
                scratch.ap(), scratch.ap(), mybir.ActivationFunctionType.Sigmoid
            )
            for r in range(NG):
                h, r4 = divmod(r, 4)
                if r == 0:
                    scalar.wait_ge(sem_q0, 16 * B_LOC)
                elif r == 1:
                    scalar.wait_ge(sem_q1, 16 * B_LOC)
                elif r == 4:
                    scalar.wait_ge(sem_h1, 16 * B_LOC)
                # in: (ip, j, c') strided read of the (c', ip, j) tile slice
                tin_v = (
                    tin[h]
                    .ap()[:, 1024 * r4 : 1024 * (r4 + 1)]
                    .rearrange("p (c ip j) -> p ip j c", c=8, ip=2)
                )
                # out: (ip, [r4], q) with q = j*8+c' contiguous
                tout_v = tout[h].ap().rearrange(
                    "p (ip r4 q) -> p ip r4 q", ip=2, r4=4
                )[:, :, r4, :]
                scalar.activation(
                    tout_v, tin_v, mybir.ActivationFunctionType.Sigmoid
                ).then_inc(sem_act, 1)

    return nc


def kernel(FV, batch_size=None, W=None, H=None, **_ignored):
    global _cached_nc, LAST_EXEC_NS
    FV = np.asarray(FV, dtype=np.float32)
    assert FV.shape == (B, 128, S, S), FV.shape

    trace = bool(os.environ.get("BASS_TRACE"))
    if trace:
        _install_trace_hook()

    if _cached_nc is None:
        _cached_nc = _build_nc()
    nc = _cached_nc

    in_maps = [{"FV": FV[k * B_LOC : (k + 1) * B_LOC]} for k in range(N_CORES)]
    res = run_bass_kernel_spmd(nc, in_maps, list(range(N_CORES)), trace=trace)
    if trace:
        LAST_EXEC_NS = res.exec_time_ns

    outs = [res.results[k]["OUT"] for k in range(N_CORES)]
    full = np.concatenate(outs, axis=0)  # [32, 512, 512]
    return full[:, None, :, :].astype(np.float32)


# revision 7
# speedup vs baseline: 1.0133x; 1.0133x over previous
"""Trainium2 Bass kernel for nn_FMNet pixel-shuffle + sigmoid.

reference:  x = FV[:, 64:, :, :]                                 # [B, 64, 64, 64]
            out[b, 8i+r, 8j+c] = sigmoid(x[b, 8r+c, i, j])       # [B, 1, 512, 512]

Per core (4 batches, pure data-parallel over batch):
  - 8 SWDGE loads (gpsimd Q7 generator) of 512 KiB: per (batch, channel-half),
    partition = (b, i2) spatial-row-pair, 512-byte contiguous HBM chunks.
    SWDGE keeps the load descriptor generation off the single shared HWDGE.
  - 8 fused ScalarE ACTIVATE(Sigmoid) ops [128 x 1024] whose strided input AP
    performs the (c', j) -> (j*8 + c') pixel-shuffle interleave in the same
    pass (measured ~2 ns/elem; DVE/GpSimd strided copies are ~4.4 ns/elem).
  - 16 HWDGE stores (SP engine, now otherwise idle) of 256 KiB: per
    (batch, r-quarter), 4 KiB contiguous HBM chunks, issued as soon as the
    two ACTs they depend on are done - keeps the store tail short.
"""

import os
import sys

if "/opt/trn_rl_repo" not in sys.path:
    sys.path.insert(0, "/opt/trn_rl_repo")

import numpy as np

import concourse.bass as bass
from concourse import mybir
from concourse.bass_utils import run_bass_kernel_spmd

N_CORES = 8
B = 32
B_LOC = B // N_CORES   # 4
H = W = 512
S = 64
NG = 8                 # channel groups (r)

LAST_EXEC_NS = None

_cached_nc = None


def _install_trace_hook():
    """Best-effort NTFF hook so BASS_TRACE=1 yields exec_time_ns."""
    try:
        import types

        import antenv

        try:
            from antenv.axon_hooks import get_axon_ntff_profile_hook  # noqa: F401

            return
        except ImportError:
            pass
        mod = types.ModuleType("antenv.axon_hooks")
        _state = {"hook": None}
        mod.set_axon_ntff_profile_hook = lambda h: _state.__setitem__("hook", h)
        mod.get_axon_ntff_profile_hook = lambda: _state["hook"]
        sys.modules["antenv.axon_hooks"] = mod
        antenv.axon_hooks = mod
        from trn_agent_boot.trn_boot import _ntff_profile_via_ctypes

        mod.set_axon_ntff_profile_hook(
            _ntff_profile_via_ctypes("/opt/axon/libaxon_pjrt.so")
        )
    except Exception:
        pass


def _build_nc():
    import contextlib

    F32 = mybir.dt.float32
    nc = bass.Bass("TRN2", num_devices=N_CORES)
    FV = nc.declare_dram_parameter("FV", [B_LOC, 128, S, S], F32, isOutput=False)
    OUT = nc.declare_dram_parameter("OUT", [B_LOC, W, H], F32, isOutput=True)

    # TIN_h[p=(b,i2), (c32, ip, j)] for channel half h (c32 = c' within half)
    tin = [nc.alloc_sbuf_tensor(f"tin{h}", [128, 4096], F32) for h in range(2)]
    # TOUT_h[p=(b,i2), (ip, r4, q)] for r half h
    tout = [nc.alloc_sbuf_tensor(f"tout{h}", [128, 4096], F32) for h in range(2)]

    fv = FV[:]
    out = OUT[:]

    scratch = nc.alloc_sbuf_tensor("scratch", [1, 8], F32)

    def store_aps(b, rq):
        """(dst, src) APs for the store of batch b, r-quarter rq."""
        h, k = divmod(rq, 2)  # tout half h, quarter k within half
        # dest rows 16*i2 + 8*ip + (2rq + r2), cols q
        dst = out[b].rearrange(
            "(i2 ip rr r2) q -> i2 ip rr (r2 q)", i2=32, ip=2, rr=4
        )[:, :, rq, :]  # [32, 2, 1024]
        src = tout[h].ap().rearrange(
            "p (ip r2 v) -> p ip r2 v", ip=2, r2=2
        )[32 * b : 32 * b + 32, :, k, :]  # [32, 2, 1024]
        return dst, src

    with contextlib.ExitStack() as stack:
        block = stack.enter_context(nc.Block())
        # one sem per channel-octant load wave (feeds ACT_g)
        sem_oct = [stack.enter_context(nc.semaphore(f"sem_o{g}")) for g in range(NG)]
        sem_act = stack.enter_context(nc.semaphore("sem_act"))
        sem_out = stack.enter_context(nc.semaphore("sem_out"))

        @block.sync
        def _(sync: bass.BassEngine):
            # 32 octant loads; ~0.63us HWDGE dispatch each self-paces against
            # the ~205 GB/s SDMA rate for 512 B descriptors
            for g in range(NG):
                h, g4 = divmod(g, 4)
                for b in range(B_LOC):
                    src = fv[b, 64 + 8 * g : 64 + 8 * g + 8]  # [8, 64, 64]
                    src = src.rearrange("c (i2 ip) j -> i2 c (ip j)", ip=2)
                    dst = tin[h].ap()[
                        32 * b : 32 * b + 32, 1024 * g4 : 1024 * (g4 + 1)
                    ]
                    sync.dma_start(out=dst, in_=src).then_inc(sem_oct[g], 16)
            # last store quarter on the (now idle) HWDGE for the short tail
            sync.wait_ge(sem_act, 8)
            for b in range(B_LOC):
                dst, src = store_aps(b, 3)
                sync.dma_start(out=dst, in_=src).then_inc(sem_out, 16)
            sync.wait_ge(sem_out, 16 * 4 * B_LOC)

        @block.scalar
        def _(scalar: bass.BassEngine):
            # dummy op to pull ACT_TABLE_LOAD (sigmoid) off the critical path
            scalar.activation(
                scratch.ap(), scratch.ap(), mybir.ActivationFunctionType.Sigmoid
            )
            for r in range(NG):
                h, r4 = divmod(r, 4)
                scalar.wait_ge(sem_oct[r], 16 * B_LOC)
                # in: (ip, j, c') strided read of the (c', ip, j) tile slice
                tin_v = (
                    tin[h]
                    .ap()[:, 1024 * r4 : 1024 * (r4 + 1)]
                    .rearrange("p (c ip j) -> p ip j c", c=8, ip=2)
                )
                # out: (ip, [r4], q) with q = j*8+c' contiguous
                tout_v = tout[h].ap().rearrange(
                    "p (ip r4 q) -> p ip r4 q", ip=2, r4=4
                )[:, :, r4, :]
                scalar.activation(
                    tout_v, tin_v, mybir.ActivationFunctionType.Sigmoid
                ).then_inc(sem_act, 1)

        @block.gpsimd
        def _(g: bass.BassEngine):
            # store quarters 0-2 via SWDGE (parallel descriptor generator)
            for rq in range(3):
                g.wait_ge(sem_act, 2 * (rq + 1))
                for b in range(B_LOC):
                    dst, src = store_aps(b, rq)
                    g.dma_start(out=dst, in_=src).then_inc(sem_out, 16)

    return nc


def kernel(FV, batch_size=None, W=None, H=None, **_ignored):
    global _cached_nc, LAST_EXEC_NS
    FV = np.asarray(FV, dtype=np.float32)
    assert FV.shape == (B, 128, S, S), FV.shape

    trace = bool(os.environ.get("BASS_TRACE"))
    if trace:
        _install_trace_hook()

    if _cached_nc is None:
        _cached_nc = _build_nc()
    nc = _cached_nc

    in_maps = [{"FV": FV[k * B_LOC : (k + 1) * B_LOC]} for k in range(N_CORES)]
    res = run_bass_kernel_spmd(nc, in_maps, list(range(N_CORES)), trace=trace)
    if trace:
        LAST_EXEC_NS = res.exec_time_ns

    outs = [res.results[k]["OUT"] for k in range(N_CORES)]
    full = np.concatenate(outs, axis=0)  # [32, 512, 512]
    return full[:, None, :, :].astype(np.float32)
